# revision 1
# baseline (speedup 1.0000x reference)
"""Neural CDE forward pass on 8 Trainium2 NeuronCores.

Strategy (data-parallel over batch, zero collectives):
  - B=32 samples -> 4 per core; weights replicated on every core.
  - times = arange(T), so the cubic-spline coefficients + tridiagonal solve
    form a CONSTANT linear map: every dX/dt value the RK4 scan needs is
    E @ x for a precomputed E [129, T].  One small on-device matmul.
  - The RK4 scan (64 steps x 4 stages) runs fully on-chip per core with
    feature-on-partition layout (z^T, h^T, f^T are [H=128, b]).
  - The dynamics are chaotic (~1e3 error amplification), so the whole
    data path stays fp32; the W2 contraction runs as 64 fp32 matmuls
    (chunk c stationary [128,128], h streaming).
  - Critical path per stage avoids z entirely:
      vf_s -> PE: ph = W1^T z_base (+) (alpha_s W1)^T vf_s   (psum accumulate)
           -> DVE relu -> PE 64 chunks -> ACT tanh
           -> DVE scalar_tensor_tensor(accum_out): vf_{s+1} = sum_c g*dX
    The z/RK4 bookkeeping (z_{i+1}, zpre = z_i + dt/6 (k1+2k2+2k3)) runs on
    the otherwise-idle GpSimd engine, off the critical path.
  - Two independent half-batch chains interleave so per-link ops are small
    and engines overlap across chains.
"""

import numpy as np

T = 128
B = 32
C = 64
H = 128
OUT = 8
NSTEPS = 64
NCORES = 8
BL = B // NCORES          # 4 samples per core
NE = 2 * NSTEPS + 1       # 129 distinct dX evaluation times
DT = float(np.float32(127.0) / np.float32(64.0))
HALF_DT = float(np.float32(0.5) * np.float32(DT))
SIXTH_DT = float(np.float32(DT) / np.float32(6.0))


def build_E():
    """E [NE, T]: dX(tau_j)[b, c] = sum_t E[j, t] x[b, t, c]."""
    diag = np.full(T, 4.0)
    diag[0] = 2.0
    diag[-1] = 2.0
    A = np.zeros((T, T))
    for i in range(T):
        A[i, i] = diag[i]
        if i + 1 < T:
            A[i, i + 1] = 1.0
            A[i + 1, i] = 1.0
    Ainv = np.linalg.inv(A)

    D = np.zeros((T - 1, T))
    for t in range(T - 1):
        D[t, t + 1] = 1.0
        D[t, t] = -1.0
    R = np.zeros((T, T))
    R[0] = 3.0 * D[0]
    for t in range(1, T - 1):
        R[t] = 3.0 * (D[t - 1] + D[t])
    R[T - 1] = 3.0 * D[T - 2]
    K = Ainv @ R  # knot = K @ path

    dt32 = np.float32(127.0) / np.float32(64.0)
    times32 = np.arange(T, dtype=np.float32)
    E = np.zeros((NE, T))
    for j in range(NE):
        i, half = divmod(j, 2)
        tau = np.float32(i) * dt32
        if half:
            tau = tau + np.float32(0.5) * dt32
        idx = int(np.clip(np.sum(tau > times32) - 1, 0, T - 2))
        frac = float(tau) - idx
        e_b = K[idx]
        e_2c = 6.0 * D[idx] - 4.0 * K[idx] - 2.0 * K[idx + 1]
        e_3d = -6.0 * D[idx] + 3.0 * (K[idx] + K[idx + 1])
        E[j] = e_b + frac * e_2c + frac * frac * e_3d
    return E.astype(np.float32)


def build_nc(nsteps=NSTEPS, nchains=2, use_b1=False, use_b2=False,
             zbook="pool", tanh_mode="one", acc_pool=False, sched="rr",
             use_bd=False):
    import concourse.bass as bass
    import concourse.tile as tile
    from concourse import bacc, mybir
    from contextlib import ExitStack

    f32 = mybir.dt.float32
    blc = BL // nchains

    nc = bacc.Bacc()
    x = nc.declare_dram_parameter("x", [BL, T, C], f32, isOutput=False)
    z0 = nc.declare_dram_parameter("z0", [BL, H], f32, isOutput=False)
    W1 = nc.declare_dram_parameter("W1", [H, 128], f32, isOutput=False)
    b1 = nc.declare_dram_parameter("b1", [128], f32, isOutput=False)
    W2 = nc.declare_dram_parameter("W2", [128, C * H], f32, isOutput=False)
    b2 = nc.declare_dram_parameter("b2", [C * H], f32, isOutput=False)
    Wd = nc.declare_dram_parameter("Wd", [H, OUT], f32, isOutput=False)
    bd = nc.declare_dram_parameter("bd", [OUT], f32, isOutput=False)
    emat = nc.declare_dram_parameter("emat", [NE, T], f32, isOutput=False)
    out = nc.declare_dram_parameter("out", [BL, OUT], f32, isOutput=True)

    # dX table rows are (b, c)-major so per-sample slices are contiguous.
    dram_dx = nc.dram_tensor("dram_dx", [NE, BL * C], f32)

    with ExitStack() as ctx:
        tc = ctx.enter_context(tile.TileContext(nc))
        singles = ctx.enter_context(tc.tile_pool(name="singles", bufs=1))
        w2pool = ctx.enter_context(tc.tile_pool(name="w2pool", bufs=1))
        prep = ctx.enter_context(tc.tile_pool(name="prep", bufs=2))
        psum_prep = ctx.enter_context(
            tc.tile_pool(name="psum_prep", bufs=1, space="PSUM"))
        psum_h = ctx.enter_context(
            tc.tile_pool(name="psum_h", bufs=1, space="PSUM"))
        psum_f = ctx.enter_context(
            tc.tile_pool(name="psum_f",
                         bufs=1 if (tanh_mode == "chalf" or nchains > 2) else 2,
                         space="PSUM"))
        hpool = ctx.enter_context(tc.tile_pool(name="hpool", bufs=2))
        gpool = ctx.enter_context(tc.tile_pool(name="gpool", bufs=2))
        spool = ctx.enter_context(tc.tile_pool(name="spool", bufs=2))
        dxpool = ctx.enter_context(tc.tile_pool(name="dxpool", bufs=8))

        # ---------------- prep: weights + spline dX table ----------------
        # Queue layout: SP carries the small fast loads (+ the per-eval dx
        # broadcasts later); the 4 MB w2stage load goes on DVE's HWDGE and
        # the dram_dx writes (which block on the spline matmul) on ACT's,
        # so neither head-of-line-blocks eval 0's inputs on SP.
        # xT[t, b, c] = x[b, t, c]
        xT = prep.tile([T, BL, C], f32, tag="xT")
        xap = x[:, :, :]
        nc.sync.dma_start(
            out=xT,
            in_=bass.AP(tensor=xap.tensor, offset=xap.offset,
                        ap=[[C, T], [T * C, BL], [1, C]]))
        # ET[t, j] = emat[j, t]
        ET = prep.tile([T, NE], f32, tag="ET")
        nc.sync.dma_start(out=ET, in_=emat.rearrange("j t -> t j"))

        # W1 stationary [k=h_in, m=h_out] is W1 exactly as stored.  The
        # vf accumulators are pre-scaled (vf_s' = alpha_s k_s) so every
        # z-reconstruction matmul uses the SAME stationary W1.
        W1sb = singles.tile([H, 128], f32)
        nc.sync.dma_start(out=W1sb, in_=W1[:, :])

        # W2 chunk-contiguous: W2sb[k, c, h] = W2[k, h*C + c]
        w2stage = w2pool.tile([128, C * H], f32, tag="w2stage")
        nc.sync.dma_start(out=w2stage, in_=W2[:, :])

        # dX[j, (b, c)] = sum_t E[j, t] * xT[t, b, c]
        pdx_pool = psum_prep if nchains <= 2 else psum_h
        pdx_a = pdx_pool.tile([128, BL * C], f32,
                              tag="pdx" if nchains <= 2 else "ph2")
        nc.tensor.matmul(out=pdx_a, lhsT=ET[:, 0:128],
                         rhs=xT.rearrange("t b c -> t (b c)"),
                         start=True, stop=True)
        pdx_b = pdx_pool.tile([1, BL * C], f32,
                              tag="pdxb" if nchains <= 2 else "ph3")
        nc.tensor.matmul(out=pdx_b, lhsT=ET[:, 128:129],
                         rhs=xT.rearrange("t b c -> t (b c)"),
                         start=True, stop=True)
        dx_a = prep.tile([128, BL * C], f32, tag="dxa")
        nc.scalar.copy(out=dx_a, in_=pdx_a)
        dx_b = prep.tile([1, BL * C], f32, tag="dxb")
        nc.scalar.copy(out=dx_b, in_=pdx_b)
        nc.sync.dma_start(out=dram_dx[0:128, :], in_=dx_a)
        nc.sync.dma_start(out=dram_dx[128:129, :], in_=dx_b)
        W2sb = singles.tile([128, C, H], f32)
        stg = w2stage.rearrange("k (h c) -> k c h", c=C)
        for q in range(8):
            sl = slice(q * 8, (q + 1) * 8)
            if q % 2 == 0:
                nc.vector.tensor_copy(out=W2sb[:, sl, :], in_=stg[:, sl, :])
            else:
                nc.scalar.copy(out=W2sb[:, sl, :], in_=stg[:, sl, :])

        if use_b1:
            b1sb = singles.tile([128, 1], f32)
            nc.sync.dma_start(out=b1sb, in_=b1[:].unsqueeze(1))
        if use_b2:
            # b2sb[h, c] = b2[h*C + c]
            b2sb = singles.tile([H, C], f32)
            nc.sync.dma_start(out=b2sb, in_=b2.rearrange("(h c) -> h c", c=C))
        Wdsb = singles.tile([H, OUT], f32)
        nc.sync.dma_start(out=Wdsb, in_=Wd[:, :])
        if use_bd:
            bdsb = singles.tile([OUT, 1], f32)
            nc.sync.dma_start(out=bdsb, in_=bd[:].unsqueeze(1))

        relu = mybir.ActivationFunctionType.Relu
        tanh = mybir.ActivationFunctionType.Tanh
        mult = mybir.AluOpType.mult
        add = mybir.AluOpType.add
        stt = nc.vector.scalar_tensor_tensor
        zeng = nc.gpsimd if zbook == "pool" else nc.vector

        # per-chain persistent state
        S = []
        for ci in range(nchains):
            st = {}
            for nm in ("zT", "zpre0", "zpre1", "u1", "u2",
                       "vf1", "vf2", "vf3", "vf4"):
                st[nm] = singles.tile([H, blc], f32, tag=f"{nm}{ci}",
                                      name=f"{nm}{ci}")
            sl = slice(ci * blc, (ci + 1) * blc)
            nc.sync.dma_start(out=st["zT"], in_=z0[sl, :].rearrange("b h -> h b"))
            S.append(st)

        def load_dx(j):
            t = dxpool.tile([128, BL, C], f32, tag="dx", bufs=8)
            nc.sync.dma_start(
                out=t,
                in_=dram_dx[j:j + 1, :].rearrange("e (b c) -> e b c", b=BL)
                    .to_broadcast([128, BL, C]))
            return t

        def vf_stage(ci, bases, dxt, alpha, vfout, w2=None):
            """One cde_func eval: vfout[:,b] = alpha * sum_c g*dX with
            h = relu(W1^T sum(bases) [+ b1]) via psum-accumulated matmuls."""
            st = S[ci]
            if w2 is None:
                w2 = W2sb
            ph = psum_h.tile([H, blc], f32, tag=f"ph{ci}")
            for j, rhs in enumerate(bases):
                nc.tensor.matmul(out=ph, lhsT=W1sb, rhs=rhs,
                                 start=(j == 0), stop=(j == len(bases) - 1))
            hf = hpool.tile([H, blc], f32, tag=f"h{ci}")
            if use_b1:
                nc.scalar.activation(out=hf, in_=ph, func=relu, bias=b1sb)
            else:
                nc.vector.tensor_scalar_max(out=hf, in0=ph, scalar1=0.0)
            if tanh_mode == "psum":
                # in-place tanh on the psum tile (blc=1: already contiguous)
                assert blc == 1 and not use_b2
                pf = psum_f.tile([H, C, blc], f32, tag=f"pf{ci}")
                for c in range(C):
                    nc.tensor.matmul(out=pf[:, c, :], lhsT=w2[:, c, :],
                                     rhs=hf, start=True, stop=True)
                nc.scalar.activation(out=pf, in_=pf, func=tanh)
                sc = spool.tile([H, C], f32, tag=f"sc{ci}")
                nc.vector.scalar_tensor_tensor(
                    out=sc, in0=pf[:, :, 0], scalar=alpha,
                    in1=dxt[:, ci * blc, :], op0=mult, op1=mult,
                    accum_out=vfout[:, 0:1])
                return
            # g[p, b, c]: per-sample slices contiguous for the reduce.
            g = gpool.tile([H, blc, C], f32, tag=f"g{ci}")
            if tanh_mode == "chalf":
                # two half-c psum tiles: tanh(half0) overlaps chunks(half1)
                hC = C // 2
                for half in range(2):
                    pf = psum_f.tile([H, hC, blc], f32, tag=f"pf{ci}h{half}")
                    for c in range(hC):
                        nc.tensor.matmul(out=pf[:, c, :],
                                         lhsT=w2[:, half * hC + c, :],
                                         rhs=hf, start=True, stop=True)
                    if use_b2:
                        stt(out=pf, in0=pf, scalar=1.0,
                            in1=b2sb[:, half * hC:(half + 1) * hC]
                                .unsqueeze(2).to_broadcast([H, hC, blc]),
                            op0=mult, op1=add)
                    gv = g[:, :, half * hC:(half + 1) * hC]
                    nc.scalar.activation(out=gv.rearrange("p b c -> p c b"),
                                         in_=pf, func=tanh)
            else:
                pf = psum_f.tile([H, C, blc], f32, tag=f"pf{ci}")
                for c in range(C):
                    nc.tensor.matmul(out=pf[:, c, :], lhsT=w2[:, c, :],
                                     rhs=hf, start=True, stop=True)
                if use_b2:
                    stt(out=pf, in0=pf, scalar=1.0,
                        in1=b2sb[:].unsqueeze(2).to_broadcast([H, C, blc]),
                        op0=mult, op1=add)
                if tanh_mode == "persample":
                    for b in range(blc):
                        nc.scalar.activation(
                            out=g[:, b, :], in_=pf[:, :, b], func=tanh)
                else:
                    nc.scalar.activation(out=g.rearrange("p b c -> p c b"),
                                         in_=pf, func=tanh)
            for b in range(blc):
                sc = spool.tile([H, C], f32, tag=f"sc{ci}b{b}")
                eng = nc.gpsimd if (acc_pool and b % 2 == 1) else nc.vector
                eng.scalar_tensor_tensor(
                    out=sc, in0=g[:, b, :], scalar=alpha,
                    in1=dxt[:, ci * blc + b, :], op0=mult, op1=mult,
                    accum_out=vfout[:, b:b + 1])

        # vf accumulators hold alpha_s * k_s:
        #   vf1 = (dt/2) k1, vf2 = (dt/2) k2, vf3 = dt k3, vf4 = (dt/6) k4
        # z_i never appears on the critical path:
        #   stage 1: h-arg = zpre_{i-1} + vf4          (= z_i)
        #   stage 2: h-arg = zpre_{i-1} + vf4 + vf1    (= z_i + dt/2 k1)
        #   stage 3: h-arg = zpre_{i-1} + vf4 + vf2
        #   stage 4: h-arg = zpre_{i-1} + vf4 + vf3
        #   zpre_i  = z_i + (vf1 + 2 vf2 + vf3)/3      (bookkeeping engine)
        zp = lambda i: f"zpre{i % 2}"

        def emit_bookkeeping_z(ci, i):
            if blc == 1:
                return  # z_i folded into the zpre update; only needed at decode
            st = S[ci]
            zeng.scalar_tensor_tensor(
                out=st["zT"], in0=st["vf4"], scalar=1.0,
                in1=st[zp(i - 1)], op0=mult, op1=add)

        def emit_bookkeeping_zpre(ci, i):
            if sched == "deprio":
                with tc.high_priority(offset=-(1 << 20)):
                    _emit_bookkeeping_zpre(ci, i)
            else:
                _emit_bookkeeping_zpre(ci, i)

        def _emit_bookkeeping_zpre(ci, i):
            # zpre_i = zpre_{i-1} + vf4 + (vf1 + 2 vf2 + vf3)/3
            st = S[ci]
            zeng.scalar_tensor_tensor(
                out=st["u1"], in0=st["vf2"], scalar=2.0, in1=st["vf1"],
                op0=mult, op1=add)
            if blc == 1:
                # vf tiles are per-partition scalars: 3-op form, no z_i
                zeng.tensor_scalar(
                    out=st["u2"], in0=st["vf3"], scalar1=st["u1"][:, 0:1],
                    scalar2=1.0 / 3.0, op0=add, op1=mult)
                if i == 0:
                    zeng.scalar_tensor_tensor(
                        out=st[zp(i)], in0=st["u2"], scalar=1.0,
                        in1=st["zT"], op0=mult, op1=add)
                else:
                    zeng.tensor_scalar(
                        out=st[zp(i)], in0=st["u2"],
                        scalar1=st["vf4"][:, 0:1],
                        scalar2=st[zp(i - 1)][:, 0:1], op0=add, op1=add)
            else:
                zeng.scalar_tensor_tensor(
                    out=st["u2"], in0=st["vf3"], scalar=1.0, in1=st["u1"],
                    op0=mult, op1=add)
                zeng.scalar_tensor_tensor(
                    out=st[zp(i)], in0=st["u2"], scalar=1.0 / 3.0,
                    in1=st["zT"], op0=mult, op1=add)

        if sched == "skew":
            # Software-pipelined emission: chain ci runs `ci` stages behind
            # chain 0, so each engine sees an evenly staggered stream
            # instead of 4-same-stage bursts.
            dxmap = {}

            def get_dx(j):
                if j not in dxmap:
                    dxmap[j] = load_dx(j)
                return dxmap[j]

            def emit(ci, pos):
                i, s = divmod(pos, 4)
                st = S[ci]
                if s == 0:
                    base = [st["zT"]] if i == 0 else \
                        [st[zp(i - 1)], st["vf4"]]
                    vf_stage(ci, base, get_dx(2 * i), HALF_DT, st["vf1"])
                    if i > 0:
                        emit_bookkeeping_z(ci, i)
                elif s == 1:
                    base = [st["zT"], st["vf1"]] if i == 0 else \
                        [st[zp(i - 1)], st["vf4"], st["vf1"]]
                    vf_stage(ci, base, get_dx(2 * i + 1), HALF_DT, st["vf2"])
                elif s == 2:
                    base = [st["zT"], st["vf2"]] if i == 0 else \
                        [st[zp(i - 1)], st["vf4"], st["vf2"]]
                    vf_stage(ci, base, get_dx(2 * i + 1), DT, st["vf3"])
                    emit_bookkeeping_zpre(ci, i)
                else:
                    base = [st["zT"], st["vf3"]] if i == 0 else \
                        [st[zp(i - 1)], st["vf4"], st["vf3"]]
                    vf_stage(ci, base, get_dx(2 * i + 2), SIXTH_DT, st["vf4"])

            total = nsteps * 4
            for t in range(total + nchains - 1):
                for ci in range(nchains):
                    pos = t - ci
                    if 0 <= pos < total:
                        emit(ci, pos)
            # fall through to decode
            nsteps_done = True
        else:
            nsteps_done = False

        def corder(s):
            # rotate which chain leads each stage burst (sched="rot")
            if sched != "rot":
                return range(nchains)
            return [(ci + s) % nchains for ci in range(nchains)]

        dx_cur = None if nsteps_done else load_dx(0)
        for i in ([] if nsteps_done else range(nsteps)):
            dx_mid = load_dx(2 * i + 1)
            dx_end = load_dx(2 * i + 2)
            for ci in corder(4 * i):
                st = S[ci]
                base = [st["zT"]] if i == 0 else [st[zp(i - 1)], st["vf4"]]
                vf_stage(ci, base, dx_cur, HALF_DT, st["vf1"])
            if i > 0:
                for ci in range(nchains):
                    emit_bookkeeping_z(ci, i)
            for ci in corder(4 * i + 1):
                st = S[ci]
                base = [st["zT"], st["vf1"]] if i == 0 else \
                    [st[zp(i - 1)], st["vf4"], st["vf1"]]
                vf_stage(ci, base, dx_mid, HALF_DT, st["vf2"])
            for ci in corder(4 * i + 2):
                st = S[ci]
                base = [st["zT"], st["vf2"]] if i == 0 else \
                    [st[zp(i - 1)], st["vf4"], st["vf2"]]
                vf_stage(ci, base, dx_mid, DT, st["vf3"])
            for ci in range(nchains):
                emit_bookkeeping_zpre(ci, i)
            for ci in corder(4 * i + 3):
                st = S[ci]
                base = [st["zT"], st["vf3"]] if i == 0 else \
                    [st[zp(i - 1)], st["vf4"], st["vf3"]]
                vf_stage(ci, base, dx_end, SIXTH_DT, st["vf4"])
            dx_cur = dx_end

        # final z and decode: out = z @ Wd + bd (one shared tile, one DMA)
        osb = prep.tile([OUT, BL], f32, tag="osb")
        for ci in range(nchains):
            st = S[ci]
            zeng.scalar_tensor_tensor(
                out=st["zT"], in0=st["vf4"], scalar=1.0,
                in1=st[zp(nsteps - 1)], op0=mult, op1=add)
            pout = psum_h.tile([OUT, blc], f32, tag=f"ph{ci}")
            nc.tensor.matmul(out=pout, lhsT=Wdsb, rhs=st["zT"],
                             start=True, stop=True)
            sl = slice(ci * blc, (ci + 1) * blc)
            if use_bd:
                nc.scalar.activation(
                    out=osb[:, sl], in_=pout,
                    func=mybir.ActivationFunctionType.Identity, bias=bdsb)
            else:
                nc.scalar.copy(out=osb[:, sl], in_=pout)
        nc.sync.dma_start(out=out.rearrange("b o -> o b"), in_=osb)

    nc.compile()
    return nc


VARIANT = (4, "dve", "one", False)  # (nchains, zbook)

_NC_CACHE = {}


def _get_nc(key):
    if key not in _NC_CACHE:
        _NC_CACHE[key] = build_nc(*key)
    return _NC_CACHE[key]


def kernel(x, z0, W1, b1, W2, b2, Wd, bd):
    from concourse.bass_utils import run_bass_kernel_spmd

    E = build_E()
    use_b1 = bool(np.any(b1))
    use_b2 = bool(np.any(b2))
    use_bd = bool(np.any(bd))
    nc = _get_nc((NSTEPS, VARIANT[0], use_b1, use_b2) + VARIANT[1:]
                 + ("rr", use_bd))
    in_maps = []
    for i in range(NCORES):
        sl = slice(i * BL, (i + 1) * BL)
        in_maps.append({
            "x": np.ascontiguousarray(x[sl], np.float32),
            "z0": np.ascontiguousarray(z0[sl], np.float32),
            "W1": np.asarray(W1, np.float32), "b1": np.asarray(b1, np.float32),
            "W2": np.asarray(W2, np.float32), "b2": np.asarray(b2, np.float32),
            "Wd": np.asarray(Wd, np.float32), "bd": np.asarray(bd, np.float32),
            "emat": E,
        })
    res = run_bass_kernel_spmd(nc, in_maps, list(range(NCORES)))
    return np.concatenate([res.results[i]["out"] for i in range(NCORES)], axis=0)

